# revision 1
# baseline (speedup 1.0000x reference)
"""ChebNet (K=2, H=16) forward on 8 Trainium2 NeuronCores.

Strategy: nodes+edges sharded by destination row across 8 cores.  Both
ChebConv layers reduce to a scalar gather/scatter over the edge list:

    s[i] = sum_{e: row[e]=i} table[col[e]]       (table = dinv*x, then dinv*g2)

Per core the edge list is bucketed by source column-block J (128 nodes),
padded to S chunks of 128 edges per bucket.  Per chunk:
  - DVE scalar_tensor_tensor: msg[e] = sum_q (iota==q_e) * YB[e, q]  (gather)
  - DVE one-hot of dest lo, DVE msg-scaled one-hot of dest hi
  - PE matmul accumulates msg into PSUM grid G[lo, hi]  (scatter-add)
Tiny per-node channel math + one AllGather (z table) between the passes.
"""

import json
import os

import numpy as np

N = 100000
NC = 8
NLOC = N // NC           # 12500 nodes per core
HIB = 98                 # dest hi blocks  (local = hi*128 + lo)
Q = 128                  # source col-block width
NJ = (N + Q - 1) // Q    # 782
GJ = 8                   # J blocks per hw-loop iteration
NJP = ((NJ + GJ - 1) // GJ) * GJ   # 784
NG = NJP // GJ           # 98 loop iterations
H = 16

_TRACE = bool(int(os.environ.get("KERNEL_TRACE", "0")))
_LAST_TRACE = {}


def _host_prep(x, edge_index, W1, b1, W2, b2):
    x = np.asarray(x, np.float32).reshape(-1)
    ei = np.asarray(edge_index)
    row = ei[0].astype(np.int64)
    col = ei[1].astype(np.int64)

    deg = np.bincount(row, minlength=N).astype(np.float32)

    core = row // NLOC
    J = col // Q
    q = (col % Q).astype(np.float32)
    loc = row % NLOC
    lo = (loc % 128).astype(np.float32)
    hi = (loc // 128).astype(np.float32)

    # per (core, J) bucket sizes -> global uniform S
    cnt = np.bincount(core * NJP + J, minlength=NC * NJP).reshape(NC, NJP)
    S = int(np.max((cnt + 127) // 128))
    C = NJP * S

    order = np.lexsort((J, core))
    J_s = J[order]
    core_s = core[order]
    # rank within bucket
    bucket_id = core_s * NJP + J_s
    starts = np.zeros(NC * NJP + 1, np.int64)
    np.cumsum(np.bincount(bucket_id, minlength=NC * NJP), out=starts[1:])
    k = np.arange(order.size, dtype=np.int64) - starts[bucket_id]

    chunk_col = J_s * S + k // 128
    part = k % 128
    flat = part * C + chunk_col

    per_core = []
    for c in range(NC):
        m = core_s == c
        qs = np.zeros(128 * C, np.float32)
        los = np.full(128 * C, 127.0, np.float32)
        his = np.full(128 * C, 97.0, np.float32)
        f = flat[m]
        qs[f] = q[order][m]
        los[f] = lo[order][m]
        his[f] = hi[order][m]

        deg_loc = np.zeros(128 * HIB, np.float32)
        x_loc = np.zeros(128 * HIB, np.float32)
        lidx = np.arange(NLOC)
        lflat = (lidx % 128) * HIB + lidx // 128
        deg_loc[lflat] = deg[c * NLOC:(c + 1) * NLOC]
        x_loc[lflat] = x[c * NLOC:(c + 1) * NLOC]

        CG = (C // NJP) * GJ
        ng = C // CG
        packed = np.empty((128, 3 * C), np.float32)
        qs2 = qs.reshape(128, ng, CG)
        los2 = los.reshape(128, ng, CG)
        his2 = his.reshape(128, ng, CG)
        pk = packed.reshape(128, ng, 3 * CG)
        pk[:, :, 0:CG] = qs2
        pk[:, :, CG:2 * CG] = los2
        pk[:, :, 2 * CG:3 * CG] = his2
        per_core.append(dict(
            stream=packed,
            deg_loc=deg_loc.reshape(128, HIB),
            x_loc=x_loc.reshape(128, HIB),
        ))

    # col-block-major global tables (node n -> [n%128, n//128]), zero padded
    x_cb = np.zeros(128 * NJP, np.float32)
    deg_cb = np.zeros(128 * NJP, np.float32)
    idx = np.arange(N)
    cbf = (idx % 128) * NJP + idx // 128
    x_cb[cbf] = x
    deg_cb[cbf] = deg
    x_cb = x_cb.reshape(128, NJP)
    deg_cb = deg_cb.reshape(128, NJP)

    params = np.zeros(81, np.float32)
    params[0:16] = np.asarray(W1, np.float32)[0, 0]
    params[16:32] = np.asarray(W1, np.float32)[1, 0]
    params[32:48] = np.asarray(b1, np.float32)
    params[48:64] = np.asarray(W2, np.float32)[0, :, 0]
    params[64:80] = np.asarray(W2, np.float32)[1, :, 0]
    params[80] = np.asarray(b2, np.float32).reshape(-1)[0]
    params = params.reshape(1, 81)

    in_maps = []
    for c in range(NC):
        d = per_core[c]
        in_maps.append({
            "stream": d["stream"],
            "x_cb": x_cb, "deg_cb": deg_cb,
            "x_loc": d["x_loc"], "deg_loc": d["deg_loc"],
            "params": params,
        })
    return S, C, in_maps


def _split_drain_waits(js: bytes) -> bytes:
    """This walrus build rejects >1 sync-wait per instruction; carry excess
    waits on preceding same-engine NoOps (engines dispatch in order)."""
    m = json.loads(js)

    def fix_block(bb):
        insts = bb.get("instructions")
        if not insts:
            return
        out = []
        for inst in insts:
            si = inst.get("sync_info") or {}
            waits = si.get("on_wait") or []
            if len(waits) > 1:
                for kk, w in enumerate(waits[:-1]):
                    carrier = {
                        "opcode": "NoOp",
                        "engine": inst.get("engine", "SP"),
                        "name": f"{inst['name']}_sw{kk}",
                        "ins": [],
                        "outs": [],
                        "sync_info": {"on_update": [], "on_wait": [w]},
                    }
                    if "debug" in inst:
                        carrier["debug"] = inst["debug"]
                    out.append(carrier)
                inst["sync_info"]["on_wait"] = [waits[-1]]
            out.append(inst)
        bb["instructions"] = out

    def walk(o):
        if isinstance(o, dict):
            if isinstance(o.get("instructions"), list):
                fix_block(o)
            for v in o.values():
                walk(v)
        elif isinstance(o, list):
            for v in o:
                walk(v)

    walk(m)
    return json.dumps(m).encode()


def _build_nc(S):
    import concourse.bass as bass
    import concourse.mybir as mybir
    import concourse.tile as tile
    from contextlib import ExitStack

    f32 = mybir.dt.float32
    i32 = mybir.dt.int32
    op = mybir.AluOpType
    C = NJP * S
    CG = GJ * S  # chunks per loop iteration

    nc = bass.Bass()
    streamd = nc.declare_dram_parameter("stream", [128, 3 * C], f32, isOutput=False)
    x_cb = nc.declare_dram_parameter("x_cb", [128, NJP], f32, isOutput=False)
    deg_cb = nc.declare_dram_parameter("deg_cb", [128, NJP], f32, isOutput=False)
    x_locp = nc.declare_dram_parameter("x_loc", [128, HIB], f32, isOutput=False)
    deg_locp = nc.declare_dram_parameter("deg_loc", [128, HIB], f32, isOutput=False)
    paramsp = nc.declare_dram_parameter("params", [1, 81], f32, isOutput=False)
    outp = nc.declare_dram_parameter("outp", [128, HIB], f32, isOutput=True)

    with ExitStack() as ctx:
        tc = ctx.enter_context(tile.TileContext(nc))
        const = ctx.enter_context(tc.tile_pool(name="const", bufs=1))
        streamp = ctx.enter_context(tc.tile_pool(name="stream", bufs=1))
        work = ctx.enter_context(tc.tile_pool(name="work", bufs=8))
        ybp = ctx.enter_context(tc.tile_pool(name="ybp", bufs=4))
        nodew = ctx.enter_context(tc.tile_pool(name="nodew", bufs=1))
        psum = ctx.enter_context(tc.tile_pool(name="psum", bufs=1, space="PSUM"))
        dram = ctx.enter_context(tc.tile_pool(name="dram", bufs=1, space="DRAM"))

        # ---------- constants ----------
        iota_i = const.tile([128, 128], i32)
        nc.gpsimd.iota(iota_i[:], pattern=[[1, 128]], base=0, channel_multiplier=0)
        iota = const.tile([128, 128], f32)
        nc.vector.tensor_copy(out=iota[:], in_=iota_i[:])
        iotaB = const.tile([128, 128], mybir.dt.bfloat16)
        nc.vector.tensor_copy(out=iotaB[:], in_=iota[:])
        ZT = const.tile([128, 128], f32)
        nc.gpsimd.memset(ZT[:], 0.0)
        P81 = const.tile([128, 81], f32)
        nc.sync.dma_start(out=P81[:], in_=paramsp[0:1, :].to_broadcast([128, 81]))

        # streams are staged per loop-iteration straight from DRAM (keeps all
        # DVE access patterns static; dynamic offsets live only on SP DMAs)

        # ---------- helpers ----------
        def newton_dinv(dst, deg_tile, F):
            m = nodew.tile([128, F], f32, name=f"nt_m{F}", tag=f"nt_m{F}")
            r0 = nodew.tile([128, F], f32, name=f"nt_r0{F}", tag=f"nt_r0{F}")
            t = nodew.tile([128, F], f32, name=f"nt_t{F}", tag=f"nt_t{F}")
            nc.vector.tensor_scalar(out=m[:], in0=deg_tile[:], scalar1=1.0,
                                    scalar2=None, op0=op.max)
            nc.scalar.activation(t[:], m[:], mybir.ActivationFunctionType.Sqrt)
            nc.vector.reciprocal(r0[:], t[:])
            nc.vector.tensor_tensor(out=t[:], in0=r0[:], in1=r0[:], op=op.mult)
            nc.vector.tensor_tensor(out=t[:], in0=t[:], in1=m[:], op=op.mult)
            nc.vector.tensor_scalar(out=t[:], in0=t[:], scalar1=-0.5,
                                    scalar2=1.5, op0=op.mult, op1=op.add)
            nc.vector.tensor_tensor(out=t[:], in0=t[:], in1=r0[:], op=op.mult)
            # mask deg==0 -> 0
            nc.vector.tensor_scalar(out=m[:], in0=deg_tile[:], scalar1=0.0,
                                    scalar2=None, op0=op.not_equal)
            nc.vector.tensor_tensor(out=dst[:], in0=t[:], in1=m[:], op=op.mult)

        # ---------- node tables ----------
        xcb_t = nodew.tile([128, NJP], f32)
        degcb_t = nodew.tile([128, NJP], f32)
        nc.sync.dma_start(out=xcb_t[:], in_=x_cb[:])
        nc.sync.dma_start(out=degcb_t[:], in_=deg_cb[:])
        dinv_cb = nodew.tile([128, NJP], f32)
        newton_dinv(dinv_cb, degcb_t, NJP)
        y_cb = nodew.tile([128, NJP], f32)
        nc.vector.tensor_tensor(out=y_cb[:], in0=dinv_cb[:], in1=xcb_t[:], op=op.mult)

        xl_t = nodew.tile([128, HIB], f32)
        degl_t = nodew.tile([128, HIB], f32)
        nc.sync.dma_start(out=xl_t[:], in_=x_locp[:])
        nc.sync.dma_start(out=degl_t[:], in_=deg_locp[:])
        dinv_loc = nodew.tile([128, HIB], f32)
        newton_dinv(dinv_loc, degl_t, HIB)

        # flat tables in DRAM (node n at offset n)
        y_flat = dram.tile([1, NJP * 128], f32)
        nc.sync.dma_start(
            out=y_flat[0:1, :].rearrange("o (j q) -> (o q) j", q=128),
            in_=y_cb[:],
        )
        z_flat = dram.tile([1, NLOC], f32)
        z_all = dram.tile([1, NJP * 128], f32)
        zrow = const.tile([1, 512], f32)
        nc.gpsimd.memset(zrow[:], 0.0)
        # zero z_all's padding tail (beyond N) so pass-2 YB loads see no garbage
        nc.sync.dma_start(out=z_all[0:1, N:NJP * 128], in_=zrow[0:1, 0:NJP * 128 - N])

        import concourse.bass as _b

        def emit_pass(table_dram, s_out):
            G = psum.tile([128, HIB], f32, tag="G")
            # clear via zero matmul
            nc.tensor.matmul(out=G[:], lhsT=ZT[:], rhs=ZT[:, :HIB],
                             start=True, stop=False)
            with tc.For_i(0, NG) as g:
                stg = streamp.tile([128, 3 * CG], f32, name="stg", tag="stg", bufs=3)
                nc.sync.dma_start(out=stg[:], in_=streamd[:, _b.ts(g, 3 * CG)])
                qg = stg[:, 0:CG]
                log = stg[:, CG:2 * CG]
                hig = stg[:, 2 * CG:3 * CG]
                yba = ybp.tile([128, GJ * 128], f32, tag="yba", name="yba")
                nc.sync.dma_start(
                    out=yba[:],
                    in_=table_dram[0:1, _b.ts(g, GJ * 128)].to_broadcast([128, GJ * 128]),
                )
                for jj in range(GJ):
                    yb = yba[:, 128 * jj:128 * (jj + 1)]
                    # wide one-hot of dest-lo for all S chunks of this J block
                    ol9 = work.tile([128, S * 128], f32, tag="ol9", name="ol9", bufs=4)
                    lsl = log[:, S * jj:S * (jj + 1)]
                    nc.vector.tensor_tensor(
                        out=_b.AP(ol9[:].tensor, ol9[:].offset,
                                  [list(ol9[:].ap[0]), [128, S], [1, 128]]),
                        in0=_b.AP(iota[:].tensor, iota[:].offset,
                                  [list(iota[:].ap[0]), [0, S], [1, 128]]),
                        in1=_b.AP(lsl.tensor, lsl.offset,
                                  [list(lsl.ap[0]), list(lsl.ap[1]), [0, 128]]),
                        op=op.is_equal,
                    )
                    for s in range(S):
                        cix = jj * S + s
                        scratch = work.tile([128, 128], f32, tag="scr", name="scr")
                        msg = work.tile([128, 1], f32, tag="msg", name="msg")
                        nc.vector.scalar_tensor_tensor(
                            out=scratch[:], in0=iota[:],
                            scalar=qg[:, cix:cix + 1], in1=yb[:],
                            op0=op.is_equal, op1=op.mult, accum_out=msg[:],
                        )
                        ol = ol9[:, 128 * s:128 * (s + 1)]
                        ohm = work.tile([128, HIB], f32, tag="ohm", name="ohm")
                        nc.vector.tensor_scalar(
                            out=ohm[:], in0=iota[:, :HIB],
                            scalar1=hig[:, cix:cix + 1], scalar2=msg[:],
                            op0=op.is_equal, op1=op.mult,
                        )
                        nc.tensor.matmul(out=G[:], lhsT=ol, rhs=ohm[:],
                                         start=False, stop=False)
            nc.tensor.matmul(out=G[:], lhsT=ZT[:], rhs=ZT[:, :HIB],
                             start=False, stop=True)
            nc.vector.tensor_copy(out=s_out[:], in_=G[:])

        # =================== pass 1 ===================
        s1 = nodew.tile([128, HIB], f32)
        emit_pass(y_flat, s1)

        # Tx1 = -dinv_loc * s1
        tx1 = nodew.tile([128, HIB], f32)
        nc.vector.scalar_tensor_tensor(out=tx1[:], in0=s1[:], scalar=-1.0,
                                       in1=dinv_loc[:], op0=op.mult, op1=op.mult)
        g2 = [nodew.tile([128, HIB], f32, name=f"g2_{i}", tag=f"g2{i}") for i in range(2)]
        p2 = [nodew.tile([128, HIB], f32, name=f"p2_{i}", tag=f"p2{i}") for i in range(2)]
        nc.gpsimd.memset(g2[0][:], 0.0)
        nc.gpsimd.memset(p2[0][:], 0.0)
        tv = nodew.tile([128, HIB], f32)
        hch = nodew.tile([128, HIB], f32)
        for ch in range(H):
            u_c = P81[:, ch:ch + 1]
            v_c = P81[:, 16 + ch:17 + ch]
            b1_c = P81[:, 32 + ch:33 + ch]
            w2a_c = P81[:, 48 + ch:49 + ch]
            w2b_c = P81[:, 64 + ch:65 + ch]
            nc.vector.tensor_scalar(out=tv[:], in0=tx1[:], scalar1=v_c,
                                    scalar2=None, op0=op.mult)
            nc.vector.scalar_tensor_tensor(out=hch[:], in0=xl_t[:], scalar=u_c,
                                           in1=tv[:], op0=op.mult, op1=op.add)
            nc.vector.tensor_scalar(out=hch[:], in0=hch[:], scalar1=b1_c,
                                    scalar2=0.0, op0=op.add, op1=op.max)
            a, b = ch % 2, 1 - ch % 2
            nc.vector.scalar_tensor_tensor(out=g2[b][:], in0=hch[:], scalar=w2b_c,
                                           in1=g2[a][:], op0=op.mult, op1=op.add)
            nc.vector.scalar_tensor_tensor(out=p2[b][:], in0=hch[:], scalar=w2a_c,
                                           in1=p2[a][:], op0=op.mult, op1=op.add)
        g2f = g2[H % 2]
        p2f = p2[H % 2]

        # z = dinv_loc * g2  -> z_flat -> allgather -> z_all
        zl = nodew.tile([128, HIB], f32)
        nc.vector.tensor_tensor(out=zl[:], in0=dinv_loc[:], in1=g2f[:], op=op.mult)
        nc.sync.dma_start(
            out=z_flat[0:1, 0:(HIB - 1) * 128].rearrange("o (h l) -> (o l) h", l=128),
            in_=zl[:, 0:HIB - 1],
        )
        nc.sync.dma_start(
            out=z_flat[0:1, (HIB - 1) * 128:NLOC],
            in_=zl[0:NLOC - (HIB - 1) * 128, HIB - 1:HIB],
        )
        nc.gpsimd.collective_compute(
            "AllGather", op.bypass,
            replica_groups=[list(range(NC))],
            ins=[z_flat[0:1, :]],
            outs=[z_all[0:1, 0:N]],
        )

        # =================== pass 2 ===================
        s2 = nodew.tile([128, HIB], f32)
        emit_pass(z_all, s2)

        o1 = nodew.tile([128, HIB], f32)
        nc.vector.scalar_tensor_tensor(out=o1[:], in0=s2[:], scalar=-1.0,
                                       in1=dinv_loc[:], op0=op.mult, op1=op.mult)
        nc.vector.tensor_tensor(out=o1[:], in0=o1[:], in1=p2f[:], op=op.add)
        nc.vector.tensor_scalar(out=o1[:], in0=o1[:], scalar1=P81[:, 80:81],
                                scalar2=None, op0=op.add)
        nc.sync.dma_start(out=outp[:], in_=o1[:])

    # patch: split multi-wait Drains for this walrus build
    orig = type(nc).to_json_bytes
    if not getattr(type(nc), "_drain_patched", False):
        def patched(self):
            return _split_drain_waits(orig(self))
        type(nc).to_json_bytes = patched
        type(nc)._drain_patched = True
    return nc


def _install_ntff_hook():
    """Recreate the missing antenv.axon_hooks shim so trace=True works."""
    import sys
    import types
    try:
        import antenv.axon_hooks  # noqa: F401
        return True
    except ImportError:
        pass
    try:
        from trn_agent_boot.trn_boot import _ntff_profile_via_ctypes
        hook = _ntff_profile_via_ctypes("/opt/axon/libaxon_pjrt.so")
        if hook is None:
            return False
        mod = types.ModuleType("antenv.axon_hooks")
        mod._hook = hook
        mod.get_axon_ntff_profile_hook = lambda: mod._hook
        mod.set_axon_ntff_profile_hook = lambda h: setattr(mod, "_hook", h)
        import antenv
        antenv.axon_hooks = mod
        sys.modules["antenv.axon_hooks"] = mod
        return True
    except Exception:
        return False


def kernel(x, edge_index, W1, b1, W2, b2):
    from concourse.bass_utils import run_bass_kernel_spmd

    S, C, in_maps = _host_prep(x, edge_index, W1, b1, W2, b2)
    nc = _build_nc(S)
    trace = _TRACE and _install_ntff_hook()
    res = run_bass_kernel_spmd(nc, in_maps, list(range(NC)), trace=trace)
    global _LAST_TRACE
    _LAST_TRACE = {
        "exec_time_ns": res.exec_time_ns,
        "profile_json": getattr(res, "profile_json", None),
    }
    out = np.concatenate(
        [res.results[c]["outp"].T.reshape(-1)[:NLOC] for c in range(NC)]
    ).astype(np.float32)
    return out.reshape(N, 1)

